# revision 1
# baseline (speedup 1.0000x reference)
"""Trainium2 Bass kernel for AdaptiveLogSoftmaxWithLoss (moe_routing).

Sharding: the three class dimensions are zero-padded and tensor-sharded
across the 8 cores (head 4002->4096, tail0 16000->16384, tail1
30257->30720), so every core runs an identical SPMD program over all 2048
samples with 1/8 of the output classes (6400 columns).

Per core:
  - hidden projections h0T=[512,2048], h1T=[256,2048] in transposed layout
    (fp8 DoubleRow GEMMs, inp scaled 16x / w1 64x), cast to bf16 (for the
    target dots) and to fp8*8 (as lhsT of the tail GEMMs),
  - logit shards computed in [sample, class] PSUM groups up to 4 banks wide
    (fp8 DoubleRow; head also fp8), one ACT exp (+accum_out, descaled via
    the activation scale) per group -> partial per-row sum-exp.  Logits are
    small by construction (|x| < ~4) so no max subtraction is needed,
  - target logits: the head uses a fused DVE (iota==rel)*logit pass on its
    PSUM group; the tails dot bf16 natural-layout hidden rows (batched XBAR
    DMA transposes of hT) against host-gathered target weight rows that are
    zeroed on non-owner cores,
  - emission order interleaves head groups with hidden0 blocks and hidden1
    blocks into the tail0 loop so the scalar engine (the exp bottleneck,
    ~13M elements/core) stays fed while the PE runs GEMMs.

Host combine: sum partials over cores, subtract the exact exp(0)=1
contribution of the zero-padded columns, lse = log(sum), gathers sum to the
single owner value, then NLL = -(head + masked tail terms) as in the
reference.  All heavy math (GEMMs, exp, reductions, gathers) runs on
device; the host only shards, pads, quantizes, and combines [N]-vectors.
"""

import numpy as np
import ml_dtypes

import concourse.bass as bass
import concourse.bacc as bacc
import concourse.mybir as mybir
import concourse.tile as tile
from concourse.bass_utils import run_bass_kernel_spmd

BF16 = ml_dtypes.bfloat16
FP8 = ml_dtypes.float8_e4m3
H_SCALE = 8.0     # h cast to fp8 at 8x
W_SCALE = 64.0    # tail w2 cast to fp8 at 64x
IN_SCALE = 16.0   # inp cast to fp8 at 16x
W1_SCALE = 64.0   # w1 / head_w cast to fp8 at 64x
HID_DESCALE = 1.0 / (IN_SCALE * W1_SCALE)
NCORES = 8
N, D = 2048, 1024
H0, H1 = 512, 256
C0, C1 = 4000, 20000
HEAD = 4002        # 4000 shortlist + 2 cluster-logit columns
HEAD_PAD = 4096    # padded so 8 cores get 512 each
T0 = 16000
T0_PAD = 16000     # divides by 8 exactly (2000 each, no padding)
T1 = 30257
T1_PAD = 30720     # padded so 8 cores get 3840 each
WH, W0, W1 = HEAD_PAD // 8, T0_PAD // 8, T1_PAD // 8   # 512, 2000, 3840
MT = N // 128                                          # 16 sample tiles
PAD_H = HEAD_PAD - HEAD   # 94 zero columns, all on core 7
PAD_0 = T0_PAD - T0       # 384 zero columns, all on core 7
PAD_1 = T1_PAD - T1       # 463 zero columns, all on core 7

# module-level knobs for test.py (harness never touches these)
TRACE = False
LAST_RESULT = None

_CACHED_NC = None


def _build_nc():
    nc = bacc.Bacc(None)
    BF = mybir.dt.bfloat16
    F8 = mybir.dt.float8e4
    F32 = mybir.dt.float32
    AX = mybir.AxisListType
    OP = mybir.AluOpType
    ACTF = mybir.ActivationFunctionType

    inpT_d = nc.dram_tensor("inpT", [128, D // 128, N], F8, kind="ExternalInput")
    w1t0_d = nc.dram_tensor("w1t0", [128, D // 128, H0], F8, kind="ExternalInput")
    w1t1_d = nc.dram_tensor("w1t1", [128, D // 128, H1], F8, kind="ExternalInput")
    hwT_d = nc.dram_tensor("hwT", [128, D // 128, WH], F8, kind="ExternalInput")
    w2t0_d = nc.dram_tensor("w2t0", [128, H0 // 128, W0], F8, kind="ExternalInput")
    w2t1_d = nc.dram_tensor("w2t1", [128, H1 // 128, W1], F8, kind="ExternalInput")
    wg0_d = nc.dram_tensor("wg0", [128, MT, H0], BF, kind="ExternalInput")
    wg1_d = nc.dram_tensor("wg1", [128, MT, H1], BF, kind="ExternalInput")
    iota_d = nc.dram_tensor("iota", [128, WH], F32, kind="ExternalInput")
    rels_d = nc.dram_tensor("rels", [128, MT, 3], F32, kind="ExternalInput")
    res_d = nc.dram_tensor("res", [128, MT, 6], F32, kind="ExternalOutput")

    with tile.TileContext(nc) as tc:
        with (
            tc.tile_pool(name="const", bufs=1) as cp,
            tc.tile_pool(name="work", bufs=3) as wp,
            tc.tile_pool(name="parts", bufs=4) as pp,
        ):
            inpT = cp.tile([128, D // 128, N], F8)
            w1t0 = cp.tile([128, D // 128, H0], F8)
            w1t1 = cp.tile([128, D // 128, H1], F8)
            hwT = cp.tile([128, D // 128, WH], F8)
            w2t0 = cp.tile([128, H0 // 128, W0], F8)
            w2t1 = cp.tile([128, H1 // 128, W1], F8)
            wg0 = cp.tile([128, MT, H0], BF)
            wg1 = cp.tile([128, MT, H1], BF)
            iota = cp.tile([128, WH], F32)
            rels = cp.tile([128, MT, 3], F32)
            h0T = cp.tile([128, H0 // 128, N], BF)
            h1T = cp.tile([128, H1 // 128, N], BF)
            h0T8 = cp.tile([128, H0 // 128, N], F8)
            h1T8 = cp.tile([128, H1 // 128, N], F8)
            h0n = cp.tile([128, MT, H0], BF)
            h1n = cp.tile([128, MT, H1], BF)
            res = cp.tile([128, MT, 6], F32)

            # loads ordered to match emission: head first, then hidden
            for kt in range(D // 128):
                nc.sync.dma_start(inpT[:, kt], inpT_d[:, kt])
                nc.sync.dma_start(hwT[:, kt], hwT_d[:, kt])
            nc.sync.dma_start(iota[:], iota_d[:])
            nc.sync.dma_start(rels[:], rels_d[:])
            nc.sync.dma_start(w1t0[:], w1t0_d[:])
            nc.sync.dma_start(w1t1[:], w1t1_d[:])
            nc.sync.dma_start(w2t0[:], w2t0_d[:])
            nc.sync.dma_start(wg0[:], wg0_d[:])
            nc.sync.dma_start(w2t1[:], w2t1_d[:])
            nc.sync.dma_start(wg1[:], wg1_d[:])

            # Front phase (head + hidden0) uses 6 one-bank slots; the
            # mid/tail phases use 2 four-bank slots.  The pools are opened
            # sequentially (the phase boundary is already data-serialized
            # on h0T8, so the pool swap costs nothing).
            fpool_cm = tc.tile_pool(name="psumF", bufs=6, space="PSUM")
            fpool = fpool_cm.__enter__()
            psp = None

            def fslot(w):
                ps = fpool.tile([128, 512], F32, tag="front", name="ps")
                return ps[:, :w]

            def pslot(w):
                ps = psp.tile([128, 2048], F32, tag="logits", name="ps")
                return ps[:, :w]

            DESCALE = 1.0 / (H_SCALE * W_SCALE)
            DR = mybir.MatmulPerfMode.DoubleRow

            def hidden_block(hT, hT8, w1, hdim, mh, alloc):
                # one h k-tile: [128 h, 2048 samples] in 512-col chunks
                for rc in range(N // 512):
                    ps = alloc(512)
                    for kt in range(0, D // 128, 2):
                        nc.tensor.matmul(
                            ps[:],
                            w1[:, kt : kt + 2, mh * 128 : (mh + 1) * 128],
                            inpT[:, kt : kt + 2, rc * 512 : (rc + 1) * 512],
                            start=(kt == 0),
                            stop=(kt + 2 >= D // 128),
                            perf_mode=DR,
                        )
                    nc.vector.tensor_scalar_mul(
                        hT[:, mh, rc * 512 : (rc + 1) * 512], ps[:], HID_DESCALE
                    )
                    nc.vector.tensor_scalar_mul(
                        hT8[:, mh, rc * 512 : (rc + 1) * 512],
                        hT[:, mh, rc * 512 : (rc + 1) * 512],
                        H_SCALE,
                    )

            def head_group(m):
                ms = slice(m * 128, (m + 1) * 128)
                ps = fslot(WH)
                for kt in range(0, D // 128, 2):
                    nc.tensor.matmul(
                        ps[:],
                        inpT[:, kt : kt + 2, ms],
                        hwT[:, kt : kt + 2, :],
                        start=(kt == 0),
                        stop=(kt + 2 >= D // 128),
                        perf_mode=DR,
                    )
                sc_e = wp.tile([128, 2048], BF, tag="sc_e")
                nc.scalar.activation(
                    sc_e[:, :WH],
                    ps[:],
                    ACTF.Exp,
                    scale=HID_DESCALE,
                    accum_out=res[:, m, 0:1],
                )
                sc_t = wp.tile([128, WH], BF, tag="sc_t")
                nc.vector.scalar_tensor_tensor(
                    out=sc_t[:],
                    in0=iota[:],
                    scalar=rels[:, m, 0:1],
                    in1=ps[:],
                    op0=OP.is_equal,
                    op1=OP.mult,
                    accum_out=res[:, m, 3:4],
                )

            def tail_group(lhsT, w2, kdim, m, gw, goff, s_ap):
                # fp8 DoubleRow GEMM group + exp/accum partial sum
                ms = slice(m * 128, (m + 1) * 128)
                ps = pslot(gw)
                nsub = kdim // 128
                for co in range(0, gw, 512):
                    cw = min(512, gw - co)
                    for kt in range(0, nsub, 2):
                        nc.tensor.matmul(
                            ps[:, co : co + cw],
                            lhsT[:, kt : kt + 2, ms],
                            w2[:, kt : kt + 2, goff + co : goff + co + cw],
                            start=(kt == 0),
                            stop=(kt + 2 >= nsub),
                            perf_mode=DR,
                        )
                sc_e = wp.tile([128, 2048], BF, tag="sc_e")
                nc.scalar.activation(
                    sc_e[:, :gw], ps[:], ACTF.Exp, scale=DESCALE, accum_out=s_ap
                )

            def transposes(hT, hn, hdim):
                # batched XBAR transpose hT[h, r] -> hn[r, h]:
                # out[p, j, q] = in[q, j*128+p]
                for kt in range(hdim // 128):
                    nc.sync.dma_start_transpose(
                        hn[:, :, kt * 128 : (kt + 1) * 128], hT[:, kt, :]
                    )

            def dot(hn, wg, hdim, m, t_ap):
                sc_d = wp.tile([128, H0], BF, tag="sc_d")
                nc.vector.scalar_tensor_tensor(
                    out=sc_d[:, :hdim],
                    in0=hn[:, m, :],
                    scalar=1.0,
                    in1=wg[:, m, :],
                    op0=OP.mult,
                    op1=OP.mult,
                    accum_out=t_ap,
                )

            # emission order feeds ACT as early as possible:
            # head -> h0 hidden -> tail0 -> h1 hidden -> tail1
            with nc.named_scope("head_hidden0"):
                for i in range(H0 // 128):
                    for m in range(4 * i, 4 * i + 4):
                        head_group(m)
                    hidden_block(h0T, h0T8, w1t0, H0, i, fslot)
            fpool_cm.__exit__(None, None, None)
            psp_cm = tc.tile_pool(name="psum", bufs=2, space="PSUM")
            psp = psp_cm.__enter__()
            transposes(h0T, h0n, H0)
            with nc.named_scope("tail0_hidden1"):
                for m in range(MT):
                    tail_group(h0T8, w2t0, H0, m, W0, 0, res[:, m, 1:2])
                    dot(h0n, wg0, H0, m, res[:, m, 4:5])
                    if m in (6, 13):
                        hidden_block(h1T, h1T8, w1t1, H1, m == 13, pslot)
            transposes(h1T, h1n, H1)
            with nc.named_scope("tail1"):
                for m in range(MT):
                    spart = pp.tile([128, 2], F32, tag="spart")
                    dot(h1n, wg1, H1, m, res[:, m, 5:6])
                    # B group first: exp on ACT without accum, sum on DVE,
                    # so the final ACT exp (A group) overlaps the B reduce
                    ms = slice(m * 128, (m + 1) * 128)
                    ps = pslot(1792)
                    for co in range(0, 1792, 512):
                        cw = min(512, 1792 - co)
                        nc.tensor.matmul(
                            ps[:, co : co + cw],
                            h1T8[:, 0:2, ms],
                            w2t1[:, 0:2, 2048 + co : 2048 + co + cw],
                            start=True,
                            stop=True,
                            perf_mode=DR,
                        )
                    sc_e = wp.tile([128, 2048], BF, tag="sc_e")
                    nc.scalar.activation(
                        sc_e[:, :1792], ps[:], ACTF.Exp, scale=DESCALE
                    )
                    nc.vector.reduce_sum(spart[:, 1:2], sc_e[:, :1792], axis=AX.X)
                    tail_group(h1T8, w2t1, H1, m, 2048, 0, spart[:, 0:1])
                    nc.vector.reduce_sum(res[:, m, 2:3], spart[:], axis=AX.X)

            psp_cm.__exit__(None, None, None)
            nc.sync.dma_start(res_d[:], res[:])

    nc.finalize()
    return nc


def _get_nc():
    global _CACHED_NC
    if _CACHED_NC is None:
        _CACHED_NC = _build_nc()
    return _CACHED_NC


def _tiled(a2d):
    """[K, F] (K multiple of 128) -> contiguous [128, K//128, F]."""
    K, F = a2d.shape
    return np.ascontiguousarray(
        a2d.reshape(K // 128, 128, F).transpose(1, 0, 2)
    )


def _pm(vec):
    """[N] -> [128, MT] with [p, m] = vec[m*128+p]."""
    return np.ascontiguousarray(vec.reshape(MT, 128).T)


def _unpm(a):
    """[128, MT] -> [N]."""
    return np.ascontiguousarray(a.T).reshape(N)


def make_in_maps(inp, tgt, head_w, t0_w1, t0_w2, t1_w1, t1_w2):
    inp = np.asarray(inp, dtype=np.float32)
    tgt = np.asarray(tgt).astype(np.int64)

    inpT = _tiled((inp.T * IN_SCALE).astype(FP8))
    w1t0 = _tiled((np.asarray(t0_w1, np.float32).T * W1_SCALE).astype(FP8))
    w1t1 = _tiled((np.asarray(t1_w1, np.float32).T * W1_SCALE).astype(FP8))

    hwT_full = np.zeros((D, HEAD_PAD), FP8)
    hwT_full[:, :HEAD] = (np.asarray(head_w, np.float32).T * W1_SCALE).astype(FP8)
    w2t0_full = (np.asarray(t0_w2, np.float32).T * W_SCALE).astype(FP8)
    w2t1_full = np.zeros((H1, T1_PAD), FP8)
    w2t1_full[:, :T1] = (np.asarray(t1_w2, np.float32).T * W_SCALE).astype(FP8)

    iota = np.broadcast_to(
        np.arange(WH, dtype=np.float32)[None, :], (128, WH)
    ).copy()

    gi = np.where(tgt < C0, tgt, np.where(tgt < C1, C0, C0 + 1))
    rel0 = tgt - C0
    rel1 = tgt - C1

    # host-gathered target weight rows (bf16, matching device operand
    # precision), zeroed on cores that don't own the target's column shard
    t0_w2_bf = np.asarray(t0_w2, np.float32).astype(BF16)
    t1_w2_bf = np.asarray(t1_w2, np.float32).astype(BF16)

    def _gather_rows(tbl, row, own):
        g = tbl[np.clip(row, 0, tbl.shape[0] - 1)]
        g[~own] = 0
        return np.ascontiguousarray(
            g.reshape(MT, 128, tbl.shape[1]).transpose(1, 0, 2)
        )

    in_maps = []
    for i in range(NCORES):
        in_maps.append(
            {
                "inpT": inpT,
                "w1t0": w1t0,
                "w1t1": w1t1,
                "hwT": _tiled(hwT_full[:, i * WH : (i + 1) * WH]),
                "w2t0": _tiled(w2t0_full[:, i * W0 : (i + 1) * W0]),
                "w2t1": _tiled(w2t1_full[:, i * W1 : (i + 1) * W1]),
                "wg0": _gather_rows(t0_w2_bf, rel0, (rel0 // W0) == i),
                "wg1": _gather_rows(t1_w2_bf, rel1, (rel1 // W1) == i),
                "iota": iota,
                "rels": np.stack(
                    [
                        _pm((gi - i * WH).astype(np.float32)),
                        _pm((rel0 - i * W0).astype(np.float32)),
                        _pm((rel1 - i * W1).astype(np.float32)),
                    ],
                    axis=2,
                ).copy(),
            }
        )
    return in_maps, tgt


def combine(results, tgt):
    """results: list of per-core {'res': [128, MT, 6]} -> final [N] f32 NLL."""
    S = np.zeros((3, N), np.float64)
    T = np.zeros((3, N), np.float64)
    for r in results:
        res = np.asarray(r["res"], np.float64)
        for c in range(3):
            S[c] += _unpm(res[:, :, c])
            T[c] += _unpm(res[:, :, 3 + c])
    S[0] -= PAD_H  # zero-padded columns contribute exp(0)=1 each (core 7)
    S[1] -= PAD_0
    S[2] -= PAD_1

    in1 = (tgt >= C0) & (tgt < C1)
    in2 = tgt >= C1
    head_term = T[0] * HID_DESCALE - np.log(S[0])
    lp0 = T[1] - np.log(S[1])
    lp1 = T[2] - np.log(S[2])
    out = head_term + np.where(in1, lp0, 0.0) + np.where(in2, lp1, 0.0)
    return (-out).astype(np.float32)


def kernel(inp, tgt, head_w, t0_w1, t0_w2, t1_w1, t1_w2):
    global LAST_RESULT
    nc = _get_nc()
    in_maps, tgt64 = make_in_maps(inp, tgt, head_w, t0_w1, t0_w2, t1_w1, t1_w2)
    out = run_bass_kernel_spmd(
        nc, in_maps, core_ids=list(range(NCORES)), trace=TRACE
    )
    LAST_RESULT = out
    return combine(out.results, tgt64)



# revision 7
# speedup vs baseline: 1.4668x; 1.4668x over previous
"""Trainium2 Bass kernel for AdaptiveLogSoftmaxWithLoss (moe_routing).

Sharding: the three class dimensions are zero-padded and tensor-sharded
across the 8 cores (head 4002->4096, tail0 16000 exact, tail1 30257->30720),
so every core runs an identical SPMD program with 1/8 of the output classes.

Row compaction: the reference discards a tail cluster's logsumexp for rows
whose target is not in that cluster, so the host compacts cluster-1 rows
(~650) and cluster-2 rows (~1230) into padded 128-row tiles; the tail GEMMs,
exps and target dots only run on those tiles (~40% less exp work, ~45% less
GEMM work than evaluating all 2048 rows for both tails).  Padded rows have
zeroed inputs (logits 0, exp 1) and are simply ignored by the host combine.

Per core:
  - head logits for all 16 sample tiles (fp8 DoubleRow GEMMs, inp scaled
    16x / head_w 64x), one ACT exp+accum (the per-row sum-exp) and a fused
    DVE (iota==rel)*logit pass (the per-row target logit) per PSUM group,
  - hidden projections h0T=[512,n1p], h1T=[256,n2p] over the compacted rows
    only, cast to bf16 (for the target dots) and fp8 (tail GEMM lhsT),
  - tail logit shards in [sample, class] PSUM groups up to 4 banks wide,
    one ACT exp (+accum_out partial sum-exp) per group; target logits dot
    bf16 natural-layout hidden rows (batched XBAR DMA transposes of hT)
    against host-gathered target weight rows zeroed on non-owner cores,
  - emission order: head groups first (ACT starts as soon as inpT+hwT land),
    hidden0 interleaved, then tail0 groups, hidden1, tail1 groups.

Host combine: sum partials over cores, subtract exp(0)=1 per zero-padded
column, lse = log(sum), scatter compacted tail terms back to their rows,
then NLL = -(head + masked tail terms) as in the reference.
"""

import numpy as np
import ml_dtypes

import concourse.bass as bass
import concourse.bacc as bacc
import concourse.mybir as mybir
import concourse.tile as tile
from concourse.bass_utils import run_bass_kernel_spmd

BF16 = ml_dtypes.bfloat16
FP8 = ml_dtypes.float8_e4m3
H_SCALE = 8.0     # h cast to fp8 at 8x
W_SCALE = 64.0    # tail w2 cast to fp8 at 64x
IN_SCALE = 16.0   # inp cast to fp8 at 16x
W1_SCALE = 64.0   # w1 / head_w cast to fp8 at 64x
HID_DESCALE = 1.0 / (IN_SCALE * W1_SCALE)
NCORES = 8
N, D = 2048, 1024
H0, H1 = 512, 256
C0, C1 = 4000, 20000
HEAD = 4002        # 4000 shortlist + 2 cluster-logit columns
HEAD_PAD = 4096    # padded so 8 cores get 512 each
T0 = 16000         # divides by 8 exactly (2000 each, no padding)
T1 = 30257
T1_PAD = 30720     # padded so 8 cores get 3840 each
WH, W0, W1 = HEAD_PAD // 8, T0 // 8, T1_PAD // 8     # 512, 2000, 3840
MT = N // 128                                        # 16 sample tiles
PAD_H = HEAD_PAD - HEAD   # 94 zero columns, all on core 7
PAD_1 = T1_PAD - T1       # 463 zero columns, all on core 7

# module-level knobs for test.py (harness never touches these)
TRACE = False
LAST_RESULT = None

_CACHED_NC = {}


def _chunks(total, step=512):
    out, o = [], 0
    while o < total:
        out.append((o, min(step, total - o)))
        o += step
    return out


def _build_nc(n1t, n2t):
    n1p, n2p = n1t * 128, n2t * 128
    NM = MT + n1t + n2t
    nc = bacc.Bacc(None)
    BF = mybir.dt.bfloat16
    F8 = mybir.dt.float8e4
    F32 = mybir.dt.float32
    AX = mybir.AxisListType
    OP = mybir.AluOpType
    ACTF = mybir.ActivationFunctionType

    inpT_d = nc.dram_tensor("inpT", [128, D // 128, N], F8, kind="ExternalInput")
    hwT_d = nc.dram_tensor("hwT", [128, D // 128, WH], F8, kind="ExternalInput")
    inpT1_d = nc.dram_tensor("inpT1", [128, D // 128, n1p], F8, kind="ExternalInput")
    inpT2_d = nc.dram_tensor("inpT2", [128, D // 128, n2p], F8, kind="ExternalInput")
    w1t0_d = nc.dram_tensor("w1t0", [128, D // 128, H0], F8, kind="ExternalInput")
    w1t1_d = nc.dram_tensor("w1t1", [128, D // 128, H1], F8, kind="ExternalInput")
    w2t0_d = nc.dram_tensor("w2t0", [128, H0 // 128, W0], F8, kind="ExternalInput")
    w2t1_d = nc.dram_tensor("w2t1", [128, H1 // 128, W1], F8, kind="ExternalInput")
    wg0_d = nc.dram_tensor("wg0", [128, max(n1t, 1), H0], BF, kind="ExternalInput")
    wg1_d = nc.dram_tensor("wg1", [128, max(n2t, 1), H1], BF, kind="ExternalInput")
    # iota [128, WH] ++ head rels [128, MT]
    misc_d = nc.dram_tensor("misc", [128, WH + MT], F32, kind="ExternalInput")
    res_d = nc.dram_tensor("res", [128, NM, 2], F32, kind="ExternalOutput")

    with tile.TileContext(nc) as tc:
        with (
            tc.tile_pool(name="const", bufs=1) as cp,
            tc.tile_pool(name="work", bufs=3) as wp,
            tc.tile_pool(name="parts", bufs=4) as pp,
        ):
            inpT = cp.tile([128, D // 128, N], F8)
            hwT = cp.tile([128, D // 128, WH], F8)
            inpT1 = cp.tile([128, D // 128, n1p], F8)
            inpT2 = cp.tile([128, D // 128, n2p], F8)
            w1t0 = cp.tile([128, D // 128, H0], F8)
            w1t1 = cp.tile([128, D // 128, H1], F8)
            w2t0 = cp.tile([128, H0 // 128, W0], F8)
            w2t1 = cp.tile([128, H1 // 128, W1], F8)
            wg0 = cp.tile([128, max(n1t, 1), H0], BF)
            wg1 = cp.tile([128, max(n2t, 1), H1], BF)
            misc = cp.tile([128, WH + MT], F32)
            h0T = cp.tile([128, H0 // 128, max(n1p, 128)], BF)
            h1T = cp.tile([128, H1 // 128, max(n2p, 128)], BF)
            h0T8 = cp.tile([128, H0 // 128, max(n1p, 128)], F8)
            h1T8 = cp.tile([128, H1 // 128, max(n2p, 128)], F8)
            h0n = cp.tile([128, max(n1t, 1), H0], BF)
            h1n = cp.tile([128, max(n2t, 1), H1], BF)
            res = cp.tile([128, NM, 2], F32)

            iota = misc[:, 0:WH]
            relH = misc[:, WH : WH + MT]

            # loads ordered to match emission: head deps first, then the
            # per-phase tensors in consumption order
            for kt in range(D // 128):
                nc.sync.dma_start(inpT[:, kt], inpT_d[:, kt])
                nc.sync.dma_start(hwT[:, kt], hwT_d[:, kt])
            nc.sync.dma_start(misc[:], misc_d[:])
            nc.sync.dma_start(w1t0[:], w1t0_d[:])
            if n1t:
                nc.sync.dma_start(inpT1[:], inpT1_d[:])
            nc.sync.dma_start(w2t0[:], w2t0_d[:])
            if n1t:
                nc.sync.dma_start(wg0[:], wg0_d[:])
            nc.sync.dma_start(w1t1[:], w1t1_d[:])
            if n2t:
                nc.sync.dma_start(inpT2[:], inpT2_d[:])
            nc.sync.dma_start(w2t1[:], w2t1_d[:])
            if n2t:
                nc.sync.dma_start(wg1[:], wg1_d[:])

            # Front phase (head + hidden) uses 6 one-bank slots; the tail
            # phases use 2 four-bank slots.  The pools are opened
            # sequentially (the phase boundary is already data-serialized
            # on h0T8, so the pool swap costs nothing).
            fpool_cm = tc.tile_pool(name="psumF", bufs=6, space="PSUM")
            fpool = fpool_cm.__enter__()
            psp = None

            def fslot(w):
                ps = fpool.tile([128, 512], F32, tag="front", name="ps")
                return ps[:, :w]

            def pslot(w):
                ps = psp.tile([128, 2048], F32, tag="logits", name="ps")
                return ps[:, :w]

            DESCALE = 1.0 / (H_SCALE * W_SCALE)
            DR = mybir.MatmulPerfMode.DoubleRow

            def hidden_block(hT, hT8, w1, inT, npad, mh, alloc):
                # one h k-tile: [128 h, npad samples] in <=512-col chunks
                for co, cw in _chunks(npad):
                    ps = alloc(cw)
                    for kt in range(0, D // 128, 2):
                        nc.tensor.matmul(
                            ps[:],
                            w1[:, kt : kt + 2, mh * 128 : (mh + 1) * 128],
                            inT[:, kt : kt + 2, co : co + cw],
                            start=(kt == 0),
                            stop=(kt + 2 >= D // 128),
                            perf_mode=DR,
                        )
                    nc.vector.tensor_scalar_mul(
                        hT[:, mh, co : co + cw], ps[:], HID_DESCALE
                    )
                    nc.vector.tensor_scalar_mul(
                        hT8[:, mh, co : co + cw], hT[:, mh, co : co + cw], H_SCALE
                    )

            def head_group(m):
                ms = slice(m * 128, (m + 1) * 128)
                ps = fslot(WH)
                for kt in range(0, D // 128, 2):
                    nc.tensor.matmul(
                        ps[:],
                        inpT[:, kt : kt + 2, ms],
                        hwT[:, kt : kt + 2, :],
                        start=(kt == 0),
                        stop=(kt + 2 >= D // 128),
                        perf_mode=DR,
                    )
                sc_e = wp.tile([128, 2048], BF, tag="sc_e")
                nc.scalar.activation(
                    sc_e[:, :WH],
                    ps[:],
                    ACTF.Exp,
                    scale=HID_DESCALE,
                    accum_out=res[:, m, 0:1],
                )
                sc_t = wp.tile([128, WH], BF, tag="sc_t")
                nc.vector.scalar_tensor_tensor(
                    out=sc_t[:],
                    in0=iota[:],
                    scalar=relH[:, m : m + 1],
                    in1=ps[:],
                    op0=OP.is_equal,
                    op1=OP.mult,
                    accum_out=res[:, m, 1:2],
                )

            def tail_group(lhsT, w2, kdim, mt, gw, goff, s_ap):
                # fp8 DoubleRow GEMM group + exp/accum partial sum
                ms = slice(mt * 128, (mt + 1) * 128)
                ps = pslot(gw)
                nsub = kdim // 128
                for co, cw in _chunks(gw):
                    for kt in range(0, nsub, 2):
                        nc.tensor.matmul(
                            ps[:, co : co + cw],
                            lhsT[:, kt : kt + 2, ms],
                            w2[:, kt : kt + 2, goff + co : goff + co + cw],
                            start=(kt == 0),
                            stop=(kt + 2 >= nsub),
                            perf_mode=DR,
                        )
                sc_e = wp.tile([128, 2048], BF, tag="sc_e")
                nc.scalar.activation(
                    sc_e[:, :gw], ps[:], ACTF.Exp, scale=DESCALE, accum_out=s_ap
                )

            def transposes(hT, hn, hdim):
                # batched XBAR transpose hT[h, r] -> hn[r, h]:
                # out[p, j, q] = in[q, j*128+p]
                for kt in range(hdim // 128):
                    nc.sync.dma_start_transpose(
                        hn[:, :, kt * 128 : (kt + 1) * 128], hT[:, kt, :]
                    )

            def dot(hn, wg, hdim, mt, t_ap):
                sc_d = wp.tile([128, H0], BF, tag="sc_d")
                nc.vector.scalar_tensor_tensor(
                    out=sc_d[:, :hdim],
                    in0=hn[:, mt, :],
                    scalar=1.0,
                    in1=wg[:, mt, :],
                    op0=OP.mult,
                    op1=OP.mult,
                    accum_out=t_ap,
                )

            # emission order feeds ACT as early as possible:
            # head (+hidden0 in PE slack) -> tail0 -> hidden1 -> tail1
            with nc.named_scope("head_hidden0"):
                for i in range(4):
                    for m in range(4 * i, 4 * i + 4):
                        head_group(m)
                    if n1t:
                        hidden_block(h0T, h0T8, w1t0, inpT1, n1p, i, fslot)
            fpool_cm.__exit__(None, None, None)
            psp_cm = tc.tile_pool(name="psum", bufs=2, space="PSUM")
            psp = psp_cm.__enter__()
            if n1t:
                transposes(h0T, h0n, H0)
            # interleave the two hidden1 k-tiles into the tail0 loop's PE
            # slack (at fixed positions when n1t is large enough)
            h1_sched = {}
            if n2t:
                if n1t >= 4:
                    h1_sched = {n1t - 3: 0, n1t - 1: 1}
                elif n1t >= 2:
                    h1_sched = {n1t - 2: 0, n1t - 1: 1}
            with nc.named_scope("tail0_hidden1"):
                for mt in range(n1t):
                    tail_group(h0T8, w2t0, H0, mt, W0, 0, res[:, MT + mt, 0:1])
                    dot(h0n, wg0, H0, mt, res[:, MT + mt, 1:2])
                    if mt in h1_sched:
                        hidden_block(h1T, h1T8, w1t1, inpT2, n2p, h1_sched[mt], pslot)
                if n2t and not h1_sched:
                    hidden_block(h1T, h1T8, w1t1, inpT2, n2p, 0, pslot)
                    hidden_block(h1T, h1T8, w1t1, inpT2, n2p, 1, pslot)
            if n2t:
                transposes(h1T, h1n, H1)
            with nc.named_scope("tail1"):
                BW = W1 - 2048  # 1792-wide B group
                for mt in range(n2t):
                    spart = pp.tile([128, 2], F32, tag="spart")
                    dot(h1n, wg1, H1, mt, res[:, MT + n1t + mt, 1:2])
                    # B group first: exp on ACT without accum, sum on DVE,
                    # so the final ACT exp (A group) overlaps the B reduce
                    ms = slice(mt * 128, (mt + 1) * 128)
                    ps = pslot(BW)
                    for co, cw in _chunks(BW):
                        nc.tensor.matmul(
                            ps[:, co : co + cw],
                            h1T8[:, 0:2, ms],
                            w2t1[:, 0:2, 2048 + co : 2048 + co + cw],
                            start=True,
                            stop=True,
                            perf_mode=DR,
                        )
                    sc_e = wp.tile([128, 2048], BF, tag="sc_e")
                    nc.scalar.activation(sc_e[:, :BW], ps[:], ACTF.Exp, scale=DESCALE)
                    nc.vector.reduce_sum(spart[:, 1:2], sc_e[:, :BW], axis=AX.X)
                    tail_group(h1T8, w2t1, H1, mt, 2048, 0, spart[:, 0:1])
                    nc.vector.reduce_sum(
                        res[:, MT + n1t + mt, 0:1], spart[:], axis=AX.X
                    )

            psp_cm.__exit__(None, None, None)
            nc.sync.dma_start(res_d[:], res[:])

    nc.finalize()
    return nc


def _get_nc(n1t, n2t):
    key = (n1t, n2t)
    if key not in _CACHED_NC:
        _CACHED_NC[key] = _build_nc(n1t, n2t)
    return _CACHED_NC[key]


def _tiled(a2d):
    """[K, F] (K multiple of 128) -> contiguous [128, K//128, F]."""
    K, F = a2d.shape
    return np.ascontiguousarray(
        a2d.reshape(K // 128, 128, F).transpose(1, 0, 2)
    )


def _pm(vec, nt):
    """[nt*128] -> [128, nt] with [p, m] = vec[m*128+p]."""
    return np.ascontiguousarray(vec.reshape(nt, 128).T)


def make_in_maps(inp, tgt, head_w, t0_w1, t0_w2, t1_w1, t1_w2):
    inp = np.asarray(inp, dtype=np.float32)
    tgt = np.asarray(tgt).astype(np.int64)

    idx1 = np.nonzero((tgt >= C0) & (tgt < C1))[0]
    idx2 = np.nonzero(tgt >= C1)[0]
    n1, n2 = len(idx1), len(idx2)
    n1t, n2t = (n1 + 127) // 128, (n2 + 127) // 128
    n1p, n2p = n1t * 128, n2t * 128

    inpq = (inp.T * IN_SCALE).astype(FP8)       # [D, N] fp8
    inpT = _tiled(inpq)

    def _compact_cols(idx, npad):
        c = np.zeros((D, npad), FP8)
        c[:, : len(idx)] = inpq[:, idx]
        return _tiled(c)

    inpT1 = _compact_cols(idx1, max(n1p, 128))[:, :, :n1p] if n1t else None
    inpT2 = _compact_cols(idx2, max(n2p, 128))[:, :, :n2p] if n2t else None

    w1t0 = _tiled((np.asarray(t0_w1, np.float32).T * W1_SCALE).astype(FP8))
    w1t1 = _tiled((np.asarray(t1_w1, np.float32).T * W1_SCALE).astype(FP8))

    hwT_full = np.zeros((D, HEAD_PAD), FP8)
    hwT_full[:, :HEAD] = (np.asarray(head_w, np.float32).T * W1_SCALE).astype(FP8)
    w2t0_full = (np.asarray(t0_w2, np.float32).T * W_SCALE).astype(FP8)
    w2t1_full = np.zeros((H1, T1_PAD), FP8)
    w2t1_full[:, :T1] = (np.asarray(t1_w2, np.float32).T * W_SCALE).astype(FP8)

    iota = np.broadcast_to(
        np.arange(WH, dtype=np.float32)[None, :], (128, WH)
    ).copy()

    gi = np.where(tgt < C0, tgt, np.where(tgt < C1, C0, C0 + 1))
    relc0 = np.zeros(n1p, np.int64)
    relc0[:n1] = tgt[idx1] - C0
    relc0[n1:] = -1
    relc1 = np.zeros(n2p, np.int64)
    relc1[:n2] = tgt[idx2] - C1
    relc1[n2:] = -1

    # host-gathered target weight rows (bf16, matching device operand
    # precision), zeroed on cores that don't own the target's column shard
    t0_w2_bf = np.asarray(t0_w2, np.float32).astype(BF16)
    t1_w2_bf = np.asarray(t1_w2, np.float32).astype(BF16)

    def _gather_rows(tbl, row, own, nt):
        g = tbl[np.clip(row, 0, tbl.shape[0] - 1)]
        g[~own] = 0
        return np.ascontiguousarray(
            g.reshape(nt, 128, tbl.shape[1]).transpose(1, 0, 2)
        )

    in_maps = []
    for i in range(NCORES):
        misc_cols = [iota, _pm((gi - i * WH).astype(np.float32), MT)]
        m = {
            "inpT": inpT,
            "hwT": _tiled(hwT_full[:, i * WH : (i + 1) * WH]),
            "w1t0": w1t0,
            "w1t1": w1t1,
            "w2t0": _tiled(w2t0_full[:, i * W0 : (i + 1) * W0]),
            "w2t1": _tiled(w2t1_full[:, i * W1 : (i + 1) * W1]),
            "misc": np.concatenate(misc_cols, axis=1),
        }
        if n1t:
            m["inpT1"] = inpT1
            m["wg0"] = _gather_rows(
                t0_w2_bf, relc0, ((relc0 // W0) == i) & (relc0 >= 0), n1t
            )
        if n2t:
            m["inpT2"] = inpT2
            m["wg1"] = _gather_rows(
                t1_w2_bf, relc1, ((relc1 // W1) == i) & (relc1 >= 0), n2t
            )
        in_maps.append(m)
    return in_maps, tgt, idx1, idx2, n1t, n2t


def combine(results, tgt, idx1, idx2, n1t, n2t):
    """per-core {'res': [128, NM, 2]} partials -> final [N] f32 NLL."""
    n1p, n2p = n1t * 128, n2t * 128
    NM = MT + n1t + n2t
    S = np.zeros((NM * 128,), np.float64)
    T = np.zeros((NM * 128,), np.float64)
    for r in results:
        res = np.asarray(r["res"], np.float64)   # [128, NM, 2]
        S += res[:, :, 0].T.reshape(-1)
        T += res[:, :, 1].T.reshape(-1)

    S_head = S[: N] - PAD_H  # zero-padded cols contribute exp(0)=1 (core 7)
    T_head = T[: N]
    head_term = T_head * HID_DESCALE - np.log(S_head)

    out = head_term.copy()
    if n1t:
        S0 = S[N : N + n1p]
        T0 = T[N : N + n1p]
        lp0 = T0 - np.log(S0)
        out[idx1] += lp0[: len(idx1)]
    if n2t:
        S1 = S[N + n1p : N + n1p + n2p] - PAD_1
        T1 = T[N + n1p : N + n1p + n2p]
        lp1 = T1 - np.log(S1)
        out[idx2] += lp1[: len(idx2)]
    return (-out).astype(np.float32)


def kernel(inp, tgt, head_w, t0_w1, t0_w2, t1_w1, t1_w2):
    global LAST_RESULT
    in_maps, tgt64, idx1, idx2, n1t, n2t = make_in_maps(
        inp, tgt, head_w, t0_w1, t0_w2, t1_w1, t1_w2
    )
    nc = _get_nc(n1t, n2t)
    out = run_bass_kernel_spmd(
        nc, in_maps, core_ids=list(range(NCORES)), trace=TRACE
    )
    LAST_RESULT = out
    return combine(out.results, tgt64, idx1, idx2, n1t, n2t)
